# revision 46
# baseline (speedup 1.0000x reference)
"""Trainium2 Bass kernel for a dense pre-LN transformer block (nn_Block_10453950398694).

Reference semantics (B=32, T=512, D=768, H=12, E=6, DFF=3072):
    xn = LN(x, g1, b1);  causal MHA(xn) -> o;  x2 = x + o@Wo + bo
    y  = LN(x2, g2, b2); out = x2 + relu(y@W1 + b1)@W2 + b2

Sharding: data-parallel over batch across 8 NeuronCores (4 batches/core).

Precision: qkv/v/MLP1 matmuls in fp8 e4m3 with DoubleRow (K=256 pairs);
scores in fp8; MLP2 / AV / Wo / permutations in bf16; fp32 PSUM everywhere;
LN stats and softmax normalization fp32.  fp8 weights are host-scaled by
WS=64 to clear the subnormal range; the scales cancel exactly on-device
(exp scale=1/WS^2 for q.k, a WS-valued aug column through 1/Z for v, W2/WS
for h1).  MLP-both-layers fp8 fails the 2e-2 gate (measured 2.1e-2), MLP1-
only is 1.5e-2.  LN affines are folded into the consumer weights host-side
(general beta1 rides a 7th k-column paired with a q ones-row; the v bias
folds into Wo's bias row; softmax row-constant terms cancel through o/Z).

Per-core dataflow, fully pipelined per 512-token batch group b:
  LN1(b) -> attention(b) -> LN2(b) -> MLP(b); mlp1(b-1) is interleaved into
  attention(b)'s three head-group exp waits, mlp2(b-1) into norm_wo(b),
  qk(b+1) is issued at g=2 so its SBUF->SBUF scatter DMAs drain during the
  norm/LN2/mlp2 window; batch 0 is covered by warm() HAM-keepalive filler.

  Engine balance (the measured wall-time levers, in priority order): the
  ACT engine must carry ONLY exp in attention windows (relu eviction runs
  on DVE via add+max, LN2's xn + evictions on ACT, LN1's on DVE, causal
  mask multiply on GpSimd, b2 as a DVE broadcast-add, not K=1 PE matmuls).

  - LN token-major (bn_stats/bn_aggr), PE-transpose to xT [d,t] (bf16 -
    fp8 transpose-mode needs elem-step-2 PSUM), plain paired-chunk copies
    on eviction, fp8 out.
  - q/k packed [D,72/84] DoubleRow matmuls, DMA-scattered to the padded
    layout: 4 heads per 128-partition group at 32-partition offsets ->
    row-tiled (tile_position) K=7 score matmuls, 4 heads concurrent in the
    PE array (the 7th row/col is the beta1 correction; LDWEIGHTS-bound
    since ldw-opt/FWL is disabled in this toolchain).
  - scoresT[s,tq] per head, causal column-trim; exp(scale=1/4096) from
    PSUM into bf16 expT (double-buffered so exp(g+1) overlaps av(g));
    diagonal 128x128 block masked by an upper-triangular multiply (GpSimd).
  - AV with stationary v_aug (v + WS column per head): 4 heads col-tiled
    concurrently into 32-col PE bands, accumulated over s-chunks with
    column-trimmed partial-N writes; band row 32j+6 is WS*Z.
  - one K=128 permutation matmul per 4-head group compacts heads into
    ps_cat[108,512]: o rows head-major at 0..71, Z rows at 96..107.  1/Z
    broadcast back to rows 0..71 via a [12,72] 0/1 matmul at partition 96;
    multiply; a DMA'd ones row (73rd) makes Wo_aug's last row add the
    (beta1-v-corrected) bias bo.
  - MLP: h1T = relu-on-DVE(W1.T @ xn2T + WS*b1) stored as WS*h1 in bf16;
    out = h1T.T @ (W2/WS) added into the residual; b2 pre-added to x2b
    after LN2; out DMA split 4-way across queues.

Queue routing: big weight loads + out stores on gpsimd-issued queues;
latency-critical x loads and qk scatters on sync queues.
"""

import os
import numpy as np
import ml_dtypes
from contextlib import ExitStack

import concourse.bass as bass
import concourse.mybir as mybir
import concourse.tile as tile
from concourse import bacc
from concourse.masks import make_upper_triangular, make_identity

BF = mybir.dt.bfloat16
F8 = mybir.dt.float8e4
F32 = mybir.dt.float32
AF = mybir.ActivationFunctionType
ALU = mybir.AluOpType
DR = mybir.MatmulPerfMode.DoubleRow
npBF = ml_dtypes.bfloat16
npF8 = ml_dtypes.float8_e4m3fn
WS = 64.0                    # fp8 weight scale (power of 2)

# problem constants (hardcoded per contract)
B, T, D, H, E = 32, 512, 768, 12, 6
DFF = 4 * D
EPS = 1e-5
NCORES = 8
BPC = B // NCORES            # 4 batches per core
TT = BPC * T                 # 2048 tokens per core
NT = TT // 128               # 16 token tiles
NDC = D // 128               # 6 d chunks
NFC = DFF // 128             # 24 dff chunks
OFFS = (0, 512, 896, 1152)   # expT column offsets per s-tile (causal-trimmed)
WID = (512, 384, 256, 128)   # expT widths per s-tile

_PROG_CACHE = {}


def build_program(taps=()):
    nc = bacc.Bacc("TRN2", target_bir_lowering=False, debug=False,
                   enable_asserts=False)
    t = {}
    x_d = nc.dram_tensor("x", [TT, D], F32, kind="ExternalInput").ap()
    wq_d = nc.dram_tensor("wq", [128, NDC, 80], F8, kind="ExternalInput").ap()
    wk_d = nc.dram_tensor("wk", [128, NDC, 96], F8, kind="ExternalInput").ap()
    wv_d = nc.dram_tensor("wv", [128, NDC, 96], F8, kind="ExternalInput").ap()
    wo_d = nc.dram_tensor("wo", [73, D], BF, kind="ExternalInput").ap()
    emat_d = nc.dram_tensor("emat", [128, 3, 108], BF, kind="ExternalInput").ap()
    cmap_d = nc.dram_tensor("cmap", [H, 72], BF, kind="ExternalInput").ap()
    w1_d = nc.dram_tensor("w1", [128, NDC, DFF], F8, kind="ExternalInput").ap()
    w2_d = nc.dram_tensor("w2", [128, NFC, D], BF, kind="ExternalInput").ap()
    b1_d = nc.dram_tensor("b1", [128, NFC], F32, kind="ExternalInput").ap()
    b2_d = nc.dram_tensor("b2r", [1, D], BF, kind="ExternalInput").ap()
    out_d = nc.dram_tensor("out", [TT, D], F32, kind="ExternalOutput").ap()

    def tap(name, shape, dtype):
        if name in taps:
            t[name] = nc.dram_tensor("tap_" + name, shape, dtype,
                                     kind="ExternalOutput").ap()
        return t.get(name)

    tap_xnT = tap("xnT", [128, NDC, 512], BF)     # b=0
    tap_x2a = tap("x2a", [TT, D], F32)
    tap_exp = tap("exp", [128, 4, 1280], BF)      # b=0, g=0
    tap_cat = tap("cat", [108, 512], F32)         # b=0
    tap_onT = tap("onT", [73, 512], BF)           # b=0

    with tile.TileContext(nc) as tc, ExitStack() as ctx:
        wpool = ctx.enter_context(tc.tile_pool(name="wpool", bufs=1))
        x2pool = ctx.enter_context(tc.tile_pool(name="x2", bufs=3))
        xnt1 = ctx.enter_context(tc.tile_pool(name="xnt1", bufs=2))
        xnt2 = ctx.enter_context(tc.tile_pool(name="xnt2", bufs=2))
        lnp = ctx.enter_context(tc.tile_pool(name="ln", bufs=2))
        stp = ctx.enter_context(tc.tile_pool(name="st", bufs=4))
        apool = ctx.enter_context(tc.tile_pool(name="attn", bufs=2))
        spool = ctx.enter_context(tc.tile_pool(name="attn_s", bufs=2))
        epool = ctx.enter_context(tc.tile_pool(name="attn_e", bufs=2))
        h1pool = ctx.enter_context(tc.tile_pool(name="h1", bufs=1))
        ppool = ctx.enter_context(tc.tile_pool(name="pp", bufs=4, space="PSUM"))
        pmlp = ctx.enter_context(tc.tile_pool(name="pm", bufs=3, space="PSUM"))
        pcat_pool = ctx.enter_context(tc.tile_pool(name="pcat", bufs=1, space="PSUM"))

        # ---- constants needed immediately ----
        mask_sb = wpool.tile([128, 128], BF)
        make_upper_triangular(nc, mask_sb[:], val=1.0, diag=True)
        ident_sb = wpool.tile([128, 128], BF)
        make_identity(nc, ident_sb[:])
        eps_sb = wpool.tile([128, 1], F32)
        nc.vector.memset(eps_sb[:], EPS)
        ones_sb = wpool.tile([1, 512], BF)
        nc.vector.memset(ones_sb[:], 1.0)
        ones8_sb = wpool.tile([1, 512], F8)
        nc.vector.memset(ones8_sb[:], 1.0)

        def load_x_fn(dst, i, r0, eng=None):
            # split the 384KB row-tile load across 4 DMA queues
            eng = eng or nc.gpsimd
            for s4 in range(4):
                eng.dma_start(dst[:, i, 192 * s4:192 * (s4 + 1)],
                              x_d[r0:r0 + 128, 192 * s4:192 * (s4 + 1)])

        # ---- prefetch first group's x ahead of the weight DMAs ----
        x2b_first = x2pool.tile([128, 4, D], F32, tag="x2b", name="x2b_0")
        for i in range(4):
            load_x_fn(x2b_first, i, 128 * i, eng=nc.sync)

        # b2 broadcast to all 128 partitions (once); the per-tile bias add
        # rides a DVE pass in the (ACT-offloaded) LN2 window instead of K=1
        # PE matmuls in the dense MLP2 window.
        b2bc_sb = wpool.tile([128, D], F32)

        def make_b2bc():
            for n0, n1 in ((0, 512), (512, 768)):
                pb = pmlp.tile([128, n1 - n0], F32, tag="pm", name=f"b2bc_{n0}")
                nc.tensor.matmul(pb[:], ones_sb[:, 0:128], b2r_sb[:, n0:n1],
                                 start=True, stop=True)
                nc.vector.tensor_copy(b2bc_sb[:, n0:n1], pb[:])

        # ---- HAM warmup/filler: dependency-free matmuls on the (idle
        # during batch 0) MLP psum pool keep the PE clock at 8/8 through
        # windows where no real PE work is ready ----
        _warm_n = [0]

        def warm(n):
            w = pmlp.tile([128, 128], F32, tag="pm",
                          name=f"warm_{_warm_n[0]}")
            _warm_n[0] += 1
            for _ in range(n):
                nc.tensor.matmul(w[:], ident_sb[:], ident_sb[:],
                                 start=True, stop=True)

        warm(64)

        # ---- weights / constants ----
        wq_sb = wpool.tile([128, NDC, 80], F8)
        wk_sb = wpool.tile([128, NDC, 96], F8)
        wv_sb = wpool.tile([128, NDC, 96], F8)
        wo_sb = wpool.tile([73, D], BF)
        emat_sb = wpool.tile([128, 3, 108], BF)
        cmap_sb = wpool.tile([108, 72], BF)
        w1_sb = wpool.tile([128, NDC, DFF], F8)
        w2_sb = wpool.tile([128, NFC, D], BF)
        b1_sb = wpool.tile([128, NFC], F32)
        b2r_sb = wpool.tile([1, D], BF)
        for sb_t, d_t in ((wq_sb, wq_d), (wk_sb, wk_d), (wv_sb, wv_d),
                          (wo_sb, wo_d), (emat_sb, emat_d), (b1_sb, b1_d),
                          (b2r_sb, b2_d)):
            nc.gpsimd.dma_start(sb_t[:], d_t[:])
        nc.gpsimd.dma_start(cmap_sb[96:108, :], cmap_d[:])
        # big MLP weights: chunked DMAs on the gpsimd-issued queues so the
        # latency-critical small DMAs (x, qk scatter) keep the sync queues
        for c in range(NDC):
            nc.gpsimd.dma_start(w1_sb[:, c, :], w1_d[:, c, :])
        for m in range(NFC):
            nc.gpsimd.dma_start(w2_sb[:, m, :], w2_d[:, m, :])

        def layernorm_group(b, x2b, xT_b, load_x, evict_act,
                            per_tile=False, warm_every=0):
            """LN over group b's 4 token tiles; writes transposed xT_b (fp8).

            The LN affine is folded into the consumer weights host-side, so
            the PSUM->SBUF eviction of each PE transpose is a plain copy
            (paired chunks, on ACT when evict_act to offload the DVE).
            """
            mv = stp.tile([128, 4, 2], F32, tag="mv")
            rstd = stp.tile([128, 4], F32, tag="rstd")
            if evict_act:
                nmr = stp.tile([128, 4], F32, tag="nmr")

            def stats_tile(i):
                if load_x:
                    load_x_fn(x2b, i, (4 * b + i) * 128)
                stats = stp.tile([128, 2, 6], F32, tag="bn")
                for s in range(2):
                    nc.vector.bn_stats(stats[:, s, :],
                                       x2b[:, i, 384 * s:384 * (s + 1)])
                nc.vector.bn_aggr(mv[:, i, :], stats[:])

            def rstd_tile(i):
                nc.scalar.activation(rstd[:, i:i + 1], mv[:, i, 1:2],
                                     AF.Sqrt, bias=eps_sb[:])
                nc.vector.reciprocal(rstd[:, i:i + 1], rstd[:, i:i + 1])
                if evict_act:
                    nc.vector.tensor_tensor(nmr[:, i:i + 1], mv[:, i, 0:1],
                                            rstd[:, i:i + 1],
                                            mybir.AluOpType.mult)
                    nc.vector.tensor_scalar_mul(nmr[:, i:i + 1],
                                                nmr[:, i:i + 1], -1.0)

            def evict_tile(i):
                xn = lnp.tile([128, D], BF, tag="xn")
                if evict_act:
                    nc.scalar.activation(xn[:], x2b[:, i, :], AF.Identity,
                                         bias=nmr[:, i:i + 1],
                                         scale=rstd[:, i:i + 1])
                else:
                    nc.vector.tensor_scalar(
                        out=xn[:], in0=x2b[:, i, :],
                        scalar1=mv[:, i, 0:1], scalar2=rstd[:, i:i + 1],
                        op0=ALU.subtract, op1=ALU.mult)
                for cp in range(NDC // 2):
                    pt = pmlp.tile([128, 2, 128], BF, tag="pm")
                    for half in range(2):
                        c = 2 * cp + half
                        nc.tensor.transpose(pt[:, half, :],
                                            xn[:, 128 * c:128 * (c + 1)],
                                            ident_sb[:])
                    dst = xT_b[:, 2 * cp:2 * cp + 2, 128 * i:128 * (i + 1)]
                    if evict_act:
                        nc.scalar.activation(dst, pt[:], AF.Copy)
                    else:
                        nc.vector.tensor_copy(dst, pt[:])

            if per_tile:
                # fully per-tile: tile i's transposes are gated only by tile
                # i's own stat chain (no in-order-stream wait on tile 3)
                for i in range(4):
                    stats_tile(i)
                    rstd_tile(i)
                    evict_tile(i)
            else:
                for i in range(4):
                    stats_tile(i)
                nc.scalar.activation(rstd[:], mv[:, :, 1], AF.Sqrt,
                                     bias=eps_sb[:])
                nc.vector.reciprocal(rstd[:], rstd[:])
                if evict_act:
                    nc.vector.tensor_tensor(nmr[:], mv[:, :, 0], rstd[:],
                                            mybir.AluOpType.mult)
                    nc.vector.tensor_scalar_mul(nmr[:], nmr[:], -1.0)
                for i in range(4):
                    evict_tile(i)
                if warm_every:
                    # batch-0 startup: fill the PE while the next tile's LN
                    # stat chain (DVE) runs
                    warm(warm_every)

        x2b_t = [None] * BPC
        xnT_t = [None] * BPC

        def ln1(b):
            if b == 0:
                x2b_t[b] = x2b_first
            else:
                x2b_t[b] = x2pool.tile([128, 4, D], F32, tag="x2b",
                                       name=f"x2b_{b}")
            xnT_t[b] = xnt1.tile([128, NDC, 512], F8, tag="xnT1", name=f"xnT_{b}")
            layernorm_group(b, x2b_t[b], xnT_t[b], load_x=(b != 0),
                            evict_act=False, per_tile=(b == 0))

        st = {}  # per-batch attention state

        def qk(b):
            # Packed projection (one [D,72] matmul per tensor), then an
            # SBUF->SBUF DMA scatter into the 32-partition-aligned padded
            # layout the row-tiled score matmuls need.
            xnT = xnT_t[b]
            qT = apool.tile([128, 3, 512], F8, tag="qT", name=f"qT_{b}")
            kT = apool.tile([128, 3, 512], F8, tag="kT", name=f"kT_{b}")
            for w_sb, dst, nm, ew in ((wq_sb, qT, "q", 6),
                                      (wk_sb, kT, "k", 7)):
                nw = 12 * ew
                p = pmlp.tile([nw, 512], F32, tag="pm", name=f"pqk_{b}_{nm}")
                for cc in range(NDC // 2):
                    nc.tensor.matmul(p[:], w_sb[:, 2 * cc:2 * cc + 2, 0:nw],
                                     xnT[:, 2 * cc:2 * cc + 2, :],
                                     start=(cc == 0), stop=(cc == NDC // 2 - 1),
                                     perf_mode=DR)
                pk = spool.tile([nw, 512], F8, tag="qkpk",
                                name=f"qkpk_{b}_{nm}")
                nc.vector.tensor_copy(pk[:], p[:])
                for g in range(3):
                    for j in range(4):
                        h = 4 * g + j
                        nc.sync.dma_start(dst[32 * j:32 * j + ew, g, :],
                                          pk[ew * h:ew * h + ew, :])
                if nm == "q":
                    # ones row per head band (pairs with k's beta-correction
                    # column); engine ops can't address partition 32j+6, DMA can
                    for g in range(3):
                        for j in range(4):
                            nc.sync.dma_start(qT[32 * j + 6:32 * j + 7, g, :],
                                              ones8_sb[:])
            st.setdefault(b, {}).update(qT=qT, kT=kT)
            st[b]["cat"] = pcat_pool.tile([108, 512], F32, tag="cat",
                                          name=f"cat_{b}")
            st[b]["rzb"] = spool.tile([108, 512], BF, tag="rzb",
                                      name=f"rzb_{b}")

        def vmm(b):
            xnT = xnT_t[b]
            vA = apool.tile([128, 4, 84], BF, tag="vA", name=f"vA_{b}")
            for si in range(4):
                p = ppool.tile([128, 84], F32, tag="pb", name=f"pv_{b}_{si}")
                for cc in range(NDC // 2):
                    nc.tensor.matmul(
                        p[:], xnT[:, 2 * cc:2 * cc + 2, si * 128:(si + 1) * 128],
                        wv_sb[:, 2 * cc:2 * cc + 2, 0:84],
                        start=(cc == 0), stop=(cc == NDC // 2 - 1),
                        perf_mode=DR)
                nc.vector.tensor_copy(vA[:, si, :], p[:])
            nc.vector.memset(
                vA[:].rearrange("p s (h e) -> p s h e", e=7)[:, :, :, 6:7], WS)
            st.setdefault(b, {})["vA"] = vA

        def scores_exp(b, g):
            qT, kT = st[b]["qT"], st[b]["kT"]
            expT = epool.tile([128, 4, 1280], BF, tag="expT", name=f"expT_{b}_{g}")
            st[b][f"expT{g}"] = expT
            for si in range(4):
                n = WID[si]
                for j in range(4):
                    pss = ppool.tile([128, 512], F32, tag="pb",
                                     name=f"pss_{b}_{g}_{si}_{j}")
                    nc.tensor.matmul(
                        pss[:, :n],
                        kT[32 * j:32 * j + 7, g, 128 * si:128 * (si + 1)],
                        qT[32 * j:32 * j + 7, g, 128 * si:512],
                        start=True, stop=True,
                        tile_position=(32 * j, 0))
                    o0 = OFFS[si]
                    nc.scalar.activation(expT[:, j, o0:o0 + n],
                                         pss[:, :n], AF.Exp,
                                         scale=1.0 / (WS * WS))
                    nc.gpsimd.tensor_mul(expT[:, j, o0:o0 + 128],
                                         expT[:, j, o0:o0 + 128],
                                         mask_sb[:])
            if tap_exp is not None and b == 0 and g == 0:
                nc.sync.dma_start(tap_exp[:], expT[:])

        def av_perm(b, g):
            # 4 heads col-tiled into the PE array concurrently (32-col bands,
            # each with its own stationary v and moving expT stream); one
            # eviction + one K=128 permutation matmul per group of 4 heads.
            vA, ps_cat = st[b]["vA"], st[b]["cat"]
            expT = st[b][f"expT{g}"]
            po4 = ppool.tile([128, 512], F32, tag="pb", name=f"po4_{b}_{g}")
            for si in range(4):
                for j in range(4):
                    h = 4 * g + j
                    nc.tensor.matmul(
                        po4[32 * j:32 * j + 7, 128 * si:512],
                        vA[:, si, 7 * h:7 * h + 7],
                        expT[:, j, OFFS[si]:OFFS[si] + WID[si]],
                        start=(si == 0), stop=(si == 3),
                        tile_position=(0, 32 * j), skip_group_check=True)
            osb4 = spool.tile([128, 512], BF, tag="osb", name=f"osb_{b}_{g}")
            nc.vector.tensor_copy(osb4[:], po4[:])
            nc.tensor.matmul(ps_cat[:], emat_sb[:, g, :], osb4[:],
                             start=(g == 0), stop=(g == 2),
                             skip_group_check=True)

        def norm_wo(b, prev=None):
            x2b, ps_cat, rzb = x2b_t[b], st[b]["cat"], st[b]["rzb"]
            if tap_cat is not None and b == 0:
                csb = spool.tile([108, 512], F32, tag="csb")
                nc.vector.tensor_copy(csb[:], ps_cat[:])
                nc.sync.dma_start(tap_cat[:], csb[:])
            with nc.allow_low_precision(reason="softmax 1/Z in bf16"):
                nc.vector.reciprocal(rzb[96:108, :], ps_cat[96:108, :])
            if prev is not None:
                mlp2_ti(prev, 0)
                mlp2_ti(prev, 1)
            else:
                warm(40)
            pbc = ppool.tile([72, 512], F32, tag="pb", name=f"pbc_{b}")
            nc.tensor.matmul(pbc[:], cmap_sb[96:108, :], rzb[96:108, :],
                             start=True, stop=True, tile_position=(96, 0))
            bc_sb = spool.tile([72, 512], BF, tag="bc", name=f"bc_{b}")
            nc.vector.tensor_copy(bc_sb[:], pbc[:])
            onT = apool.tile([73, 512], BF, tag="onT", name=f"onT_{b}")
            nc.vector.tensor_mul(onT[0:72, :], ps_cat[0:72, :], bc_sb[:])
            nc.sync.dma_start(onT[72:73, :], ones_sb[:])
            if tap_onT is not None and b == 0:
                nc.sync.dma_start(tap_onT[:], onT[:])
            if prev is not None:
                mlp2_ti(prev, 2)
            for ti in range(4):
                pa = ppool.tile([128, 512], F32, tag="pb", name=f"pwa_{b}_{ti}")
                pb2 = ppool.tile([128, 256], F32, tag="pb", name=f"pwb_{b}_{ti}")
                nc.tensor.matmul(pa[:], onT[:, 128 * ti:128 * (ti + 1)],
                                 wo_sb[:, 0:512], start=True, stop=True)
                nc.tensor.matmul(pb2[:], onT[:, 128 * ti:128 * (ti + 1)],
                                 wo_sb[:, 512:768], start=True, stop=True)
                nc.vector.tensor_add(x2b[:, ti, 0:512], pa[:],
                                     x2b[:, ti, 0:512])
                nc.vector.tensor_add(x2b[:, ti, 512:768], pb2[:],
                                     x2b[:, ti, 512:768])
            if prev is not None:
                mlp2_ti(prev, 3)
            if tap_x2a is not None:
                for i in range(4):
                    r0 = (4 * b + i) * 128
                    nc.sync.dma_start(tap_x2a[r0:r0 + 128, :], x2b[:, i, :])

        def ln2(b):
            xn2T = xnt2.tile([128, NDC, 512], F8, tag="xnT2", name=f"xn2T_{b}")
            st[b]["xn2T"] = xn2T
            layernorm_group(b, x2b_t[b], xn2T, load_x=False,
                            evict_act=True, per_tile=True)
            # after LN2 consumed x2b, pre-add b2 so mlp2's eviction is a
            # plain residual add
            for i in range(4):
                nc.vector.tensor_add(x2b_t[b][:, i, :], x2b_t[b][:, i, :],
                                     b2bc_sb[:])

        def mlp1_part(b, mlo, mhi, act=False):
            xn2T = st[b]["xn2T"]
            if "h1T" not in st[b]:
                st[b]["h1T"] = h1pool.tile([128, NFC, 512], BF, tag="h1T",
                                           name=f"h1T_{b}")
            h1T = st[b]["h1T"]
            for m in range(mlo, mhi):
                p = pmlp.tile([128, 512], F32, tag="pm", name=f"pm1_{b}_{m}")
                for cc in range(NDC // 2):
                    nc.tensor.matmul(
                        p[:], w1_sb[:, 2 * cc:2 * cc + 2, 128 * m:128 * (m + 1)],
                        xn2T[:, 2 * cc:2 * cc + 2, :],
                        start=(cc == 0), stop=(cc == NDC // 2 - 1),
                        perf_mode=DR)
                if act:
                    nc.scalar.activation(h1T[:, m, :], p[:], AF.Relu,
                                         bias=b1_sb[:, m:m + 1])
                else:
                    # relu on DVE: ACT is saturated by exp in this window
                    nc.vector.tensor_scalar(
                        out=h1T[:, m, :], in0=p[:],
                        scalar1=b1_sb[:, m:m + 1], scalar2=0.0,
                        op0=ALU.add, op1=ALU.max)

        def mlp2_ti(b, ti):
            x2b, h1T = x2b_t[b], st[b]["h1T"]
            if True:
                tix = 4 * b + ti
                pa = pmlp.tile([128, 512], F32, tag="pm", name=f"p2a_{b}_{ti}")
                pb2 = pmlp.tile([128, 256], F32, tag="pm", name=f"p2b_{b}_{ti}")
                for m in range(NFC):
                    nc.tensor.matmul(pa[:],
                                     h1T[:, m, 128 * ti:128 * (ti + 1)],
                                     w2_sb[:, m, 0:512],
                                     start=(m == 0), stop=(m == NFC - 1))
                    nc.tensor.matmul(pb2[:],
                                     h1T[:, m, 128 * ti:128 * (ti + 1)],
                                     w2_sb[:, m, 512:768],
                                     start=(m == 0), stop=(m == NFC - 1))
                nc.vector.tensor_add(x2b[:, ti, 0:512], pa[:],
                                     x2b[:, ti, 0:512])
                nc.vector.tensor_add(x2b[:, ti, 512:768], pb2[:],
                                     x2b[:, ti, 512:768])
                for s4 in range(4):
                    nc.gpsimd.dma_start(
                        out_d[128 * tix:128 * (tix + 1),
                              192 * s4:192 * (s4 + 1)],
                        x2b[:, ti, 192 * s4:192 * (s4 + 1)])

        # ---- software-pipelined schedule: MLP(b-1) and the next batch's
        # LN1/QKV interleaved into attention(b) so the PE in-order stream
        # always has independent matmul work during exp/DVE waits ----
        ln1(0)
        qk(0)
        for b in range(BPC):
            if b > 1:
                vmm(b)
            elif b == 0:
                make_b2bc()
            if tap_xnT is not None and b == 0:
                nc.sync.dma_start(tap_xnT[:], xnT_t[0][:])
            if b + 1 < BPC:
                ln1(b + 1)
            for g in range(3):
                scores_exp(b, g)
                if b >= 1:
                    mlp1_part(b - 1, 8 * g, 8 * (g + 1))
                else:
                    warm(64)
                    if g == 0:
                        vmm(0)
                    elif g == 1:
                        vmm(1)
                if g == 2 and b + 1 < BPC:
                    # next batch's projections: the scatter DMAs drain during
                    # the norm/ln2/mlp2 window instead of stalling scores
                    qk(b + 1)
                av_perm(b, g)
            norm_wo(b, prev=(b - 1 if b >= 1 else None))
            ln2(b)
        mlp1_part(BPC - 1, 0, NFC, act=True)
        for ti in range(4):
            mlp2_ti(BPC - 1, ti)

    nc.compile()
    return nc, t


def prepare_inputs(inputs):
    """Host-side: cast/pad/reshape weights into the kernel's layouts.

    fp8(e4m3) weights are pre-scaled by WS=64 to clear the subnormal range
    (sigma≈0.02 raw); the scales cancel exactly on-device: q·k through the
    exp scale=1/WS^2, v through the WS-valued aug column and 1/Z, W1
    through W2/WS (h1 is stored as WS·relu-units).
    """
    f = lambda k: np.asarray(inputs[k], np.float32)
    Wq, Wk, Wv, Wo = f("Wq"), f("Wk"), f("Wv"), f("Wo")
    g1v, be1v = f("g1"), f("beta1")
    g2v, be2v = f("g2"), f("beta2")
    cast = lambda a: np.ascontiguousarray(a.astype(npBF))
    cast8 = lambda a: np.ascontiguousarray(
        np.clip(a, -240.0, 240.0).astype(npF8))

    # LN affines folded into the consuming weights (z = normalized tokens):
    #   xn@W = z@(diag(g1)W) + beta1@W.  The q-side bias only shifts each
    #   softmax row by a constant (exp(c) cancels through o/Z); the k-side
    #   bias term bq.k~[s] rides a 7th k column (paired with a q ones row);
    #   the v bias passes through softmax intact -> folded into Wo's bias.
    Wq_eff = Wq * g1v[None, :, None]
    Wk_eff = Wk * g1v[None, :, None]
    Wv_eff = Wv * g1v[None, :, None]

    def qk_pack(W, scale):
        # [H, D, E] -> [D, H*E] packed columns, tiled to [128, NDC, 80]
        # (padded to 80 so the fp8 DoubleRow pair-stride is 16B-aligned)
        Wp = np.zeros((D, 80), np.float32)
        Wp[:, 0:H * E] = np.transpose(W, (1, 0, 2)).reshape(D, H * E) * scale
        return cast8(Wp.reshape(NDC, 128, 80).transpose(1, 0, 2))

    wq = qk_pack(Wq_eff, WS * E ** -0.5)
    Wk_aug = np.zeros((D, 96), np.float32)
    for h in range(H):
        bq_h = be1v @ Wq[h]
        Wk_aug[:, 7 * h:7 * h + 6] = Wk_eff[h] * WS
        Wk_aug[:, 7 * h + 6] = (Wk_eff[h] @ bq_h) * (WS * WS * E ** -0.5)
    wk = cast8(Wk_aug.reshape(NDC, 128, 96).transpose(1, 0, 2))
    Wv_aug = np.zeros((D, 96), np.float32)
    for h in range(H):
        Wv_aug[:, 7 * h:7 * h + 6] = Wv_eff[h] * WS
    wv = cast8(Wv_aug.reshape(NDC, 128, 96).transpose(1, 0, 2))
    wo = np.zeros((73, D), np.float32)
    wo[0:72] = Wo
    bv_cat = np.concatenate([be1v @ Wv[h] for h in range(H)])
    wo[72] = f("bo") + bv_cat @ Wo
    # emat: [128, 3, 108] 0/1 permutation; row 32j+e of group g routes
    # head (4g+j)'s o row e -> packed row 6h+e, and row 32j+6 routes the
    # softmax denominator Z -> row 96+h.
    emat = np.zeros((128, 3, 108), np.float32)
    cmap = np.zeros((H, 72), np.float32)
    for h in range(H):
        g, j = divmod(h, 4)
        for e in range(6):
            emat[32 * j + e, g, 6 * h + e] = 1.0
            cmap[h, 6 * h + e] = 1.0
        emat[32 * j + 6, g, 96 + h] = 1.0
    W1_eff = f("W1") * g2v[:, None]
    b1_eff = be2v @ f("W1") + f("b1")
    w1 = cast8((W1_eff * WS).reshape(NDC, 128, DFF).transpose(1, 0, 2))
    w2 = cast((f("W2") / WS).reshape(NFC, 128, D).transpose(1, 0, 2))
    b1 = np.ascontiguousarray((b1_eff * WS).reshape(NFC, 128).T)
    shared = dict(wq=wq, wk=wk, wv=wv, wo=cast(wo), emat=cast(emat),
                  cmap=cast(cmap), w1=w1, w2=w2, b1=b1,
                  b2r=cast(f("b2").reshape(1, D)))
    x = f("x")
    in_maps = []
    for c in range(NCORES):
        m = dict(shared)
        m["x"] = np.ascontiguousarray(
            x[c * BPC:(c + 1) * BPC].reshape(TT, D))
        in_maps.append(m)
    return in_maps


def kernel(**inputs):
    from concourse.bass_utils import run_bass_kernel_spmd
    key = "prog"
    if key not in _PROG_CACHE:
        _PROG_CACHE[key] = build_program()
    nc, _ = _PROG_CACHE[key]
    in_maps = prepare_inputs(inputs)
    trace = bool(int(os.environ.get("KERNEL_TRACE", "0")))
    res = run_bass_kernel_spmd(nc, in_maps, list(range(NCORES)), trace=trace)
    if trace and res.exec_time_ns is not None:
        print(f"HW exec time: {res.exec_time_ns} ns")
        _PROG_CACHE["last_exec_ns"] = res.exec_time_ns
        _PROG_CACHE["last_results"] = res
    out = np.empty((B, T, D), np.float32)
    for c in range(NCORES):
        out[c * BPC:(c + 1) * BPC] = res.results[c]["out"].reshape(BPC, T, D)
    return out



# revision 47
# speedup vs baseline: 1.0035x; 1.0035x over previous
"""Trainium2 Bass kernel for a dense pre-LN transformer block (nn_Block_10453950398694).

Reference semantics (B=32, T=512, D=768, H=12, E=6, DFF=3072):
    xn = LN(x, g1, b1);  causal MHA(xn) -> o;  x2 = x + o@Wo + bo
    y  = LN(x2, g2, b2); out = x2 + relu(y@W1 + b1)@W2 + b2

Sharding: data-parallel over batch across 8 NeuronCores (4 batches/core).

Precision: qkv/v/MLP1 matmuls in fp8 e4m3 with DoubleRow (K=256 pairs);
scores in fp8; MLP2 / AV / Wo / permutations in bf16; fp32 PSUM everywhere;
LN stats and softmax normalization fp32.  fp8 weights are host-scaled by
WS=64 to clear the subnormal range; the scales cancel exactly on-device
(exp scale=1/WS^2 for q.k, a WS-valued aug column through 1/Z for v, W2/WS
for h1).  MLP-both-layers fp8 fails the 2e-2 gate (measured 2.1e-2), MLP1-
only is 1.5e-2.  LN affines are folded into the consumer weights host-side
(general beta1 rides a 7th k-column paired with a q ones-row; the v bias
folds into Wo's bias row; softmax row-constant terms cancel through o/Z).

Per-core dataflow, fully pipelined per 512-token batch group b:
  LN1(b) -> attention(b) -> LN2(b) -> MLP(b); mlp1(b-1) is interleaved into
  attention(b)'s three head-group exp waits, mlp2(b-1) into norm_wo(b),
  qk(b+1) is issued at g=2 so its SBUF->SBUF scatter DMAs drain during the
  norm/LN2/mlp2 window; batch 0 is covered by warm() HAM-keepalive filler.

  Engine balance (the measured wall-time levers, in priority order): the
  ACT engine must carry ONLY exp in attention windows (relu eviction runs
  on DVE via add+max, LN2's xn + evictions on ACT, LN1's on DVE, causal
  mask multiply on GpSimd, b2 as a DVE broadcast-add, not K=1 PE matmuls).

  - LN token-major (bn_stats/bn_aggr), PE-transpose to xT [d,t] (bf16 -
    fp8 transpose-mode needs elem-step-2 PSUM), plain paired-chunk copies
    on eviction, fp8 out.
  - q/k packed [D,72/84] DoubleRow matmuls, DMA-scattered to the padded
    layout: 4 heads per 128-partition group at 32-partition offsets ->
    row-tiled (tile_position) K=7 score matmuls, 4 heads concurrent in the
    PE array (the 7th row/col is the beta1 correction; LDWEIGHTS-bound
    since ldw-opt/FWL is disabled in this toolchain).
  - scoresT[s,tq] per head, causal column-trim; exp(scale=1/4096) from
    PSUM into bf16 expT (double-buffered so exp(g+1) overlaps av(g));
    diagonal 128x128 block masked by an upper-triangular multiply (GpSimd).
  - AV with stationary v_aug (v + WS column per head): 4 heads col-tiled
    concurrently into 32-col PE bands, accumulated over s-chunks with
    column-trimmed partial-N writes; band row 32j+6 is WS*Z.
  - one K=128 permutation matmul per 4-head group compacts heads into
    ps_cat[108,512]: o rows head-major at 0..71, Z rows at 96..107.  1/Z
    broadcast back to rows 0..71 via a [12,72] 0/1 matmul at partition 96;
    multiply; a DMA'd ones row (73rd) makes Wo_aug's last row add the
    (beta1-v-corrected) bias bo.
  - MLP: h1T = relu-on-DVE(W1.T @ xn2T + WS*b1) stored as WS*h1 in bf16;
    out = h1T.T @ (W2/WS) added into the residual; b2 pre-added to x2b
    after LN2; out DMA split 4-way across queues.

Queue routing: big weight loads + out stores on gpsimd-issued queues;
latency-critical x loads and qk scatters on sync queues.
"""

import os
import numpy as np
import ml_dtypes
from contextlib import ExitStack

import concourse.bass as bass
import concourse.mybir as mybir
import concourse.tile as tile
from concourse import bacc
from concourse.masks import make_upper_triangular, make_identity

BF = mybir.dt.bfloat16
F8 = mybir.dt.float8e4
F32 = mybir.dt.float32
AF = mybir.ActivationFunctionType
ALU = mybir.AluOpType
DR = mybir.MatmulPerfMode.DoubleRow
npBF = ml_dtypes.bfloat16
npF8 = ml_dtypes.float8_e4m3fn
WS = 64.0                    # fp8 weight scale (power of 2)

# problem constants (hardcoded per contract)
B, T, D, H, E = 32, 512, 768, 12, 6
DFF = 4 * D
EPS = 1e-5
NCORES = 8
BPC = B // NCORES            # 4 batches per core
TT = BPC * T                 # 2048 tokens per core
NT = TT // 128               # 16 token tiles
NDC = D // 128               # 6 d chunks
NFC = DFF // 128             # 24 dff chunks
OFFS = (0, 512, 896, 1152)   # expT column offsets per s-tile (causal-trimmed)
WID = (512, 384, 256, 128)   # expT widths per s-tile

_PROG_CACHE = {}


def build_program(taps=()):
    nc = bacc.Bacc("TRN2", target_bir_lowering=False, debug=False,
                   enable_asserts=False)
    t = {}
    x_d = nc.dram_tensor("x", [TT, D], F32, kind="ExternalInput").ap()
    wq_d = nc.dram_tensor("wq", [128, NDC, 80], F8, kind="ExternalInput").ap()
    wk_d = nc.dram_tensor("wk", [128, NDC, 96], F8, kind="ExternalInput").ap()
    wv_d = nc.dram_tensor("wv", [128, NDC, 96], F8, kind="ExternalInput").ap()
    wo_d = nc.dram_tensor("wo", [73, D], BF, kind="ExternalInput").ap()
    emat_d = nc.dram_tensor("emat", [128, 3, 108], BF, kind="ExternalInput").ap()
    cmap_d = nc.dram_tensor("cmap", [H, 72], BF, kind="ExternalInput").ap()
    w1_d = nc.dram_tensor("w1", [128, NDC, DFF], F8, kind="ExternalInput").ap()
    w2_d = nc.dram_tensor("w2", [128, NFC, D], BF, kind="ExternalInput").ap()
    b1_d = nc.dram_tensor("b1", [128, NFC], F32, kind="ExternalInput").ap()
    b2_d = nc.dram_tensor("b2r", [1, D], BF, kind="ExternalInput").ap()
    out_d = nc.dram_tensor("out", [TT, D], F32, kind="ExternalOutput").ap()

    def tap(name, shape, dtype):
        if name in taps:
            t[name] = nc.dram_tensor("tap_" + name, shape, dtype,
                                     kind="ExternalOutput").ap()
        return t.get(name)

    tap_xnT = tap("xnT", [128, NDC, 512], BF)     # b=0
    tap_x2a = tap("x2a", [TT, D], F32)
    tap_exp = tap("exp", [128, 4, 1280], BF)      # b=0, g=0
    tap_cat = tap("cat", [108, 512], F32)         # b=0
    tap_onT = tap("onT", [73, 512], BF)           # b=0

    with tile.TileContext(nc) as tc, ExitStack() as ctx:
        wpool = ctx.enter_context(tc.tile_pool(name="wpool", bufs=1))
        x2pool = ctx.enter_context(tc.tile_pool(name="x2", bufs=3))
        xnt1 = ctx.enter_context(tc.tile_pool(name="xnt1", bufs=2))
        xnt2 = ctx.enter_context(tc.tile_pool(name="xnt2", bufs=2))
        lnp = ctx.enter_context(tc.tile_pool(name="ln", bufs=2))
        stp = ctx.enter_context(tc.tile_pool(name="st", bufs=4))
        apool = ctx.enter_context(tc.tile_pool(name="attn", bufs=2))
        spool = ctx.enter_context(tc.tile_pool(name="attn_s", bufs=2))
        epool = ctx.enter_context(tc.tile_pool(name="attn_e", bufs=2))
        h1pool = ctx.enter_context(tc.tile_pool(name="h1", bufs=1))
        ppool = ctx.enter_context(tc.tile_pool(name="pp", bufs=4, space="PSUM"))
        pmlp = ctx.enter_context(tc.tile_pool(name="pm", bufs=3, space="PSUM"))
        pcat_pool = ctx.enter_context(tc.tile_pool(name="pcat", bufs=1, space="PSUM"))

        # ---- constants needed immediately ----
        mask_sb = wpool.tile([128, 128], BF)
        make_upper_triangular(nc, mask_sb[:], val=1.0, diag=True)
        ident_sb = wpool.tile([128, 128], BF)
        make_identity(nc, ident_sb[:])
        eps_sb = wpool.tile([128, 1], F32)
        nc.vector.memset(eps_sb[:], EPS)
        ones_sb = wpool.tile([1, 512], BF)
        nc.vector.memset(ones_sb[:], 1.0)
        ones8_sb = wpool.tile([1, 512], F8)
        nc.vector.memset(ones8_sb[:], 1.0)

        def load_x_fn(dst, i, r0, eng=None):
            # split the 384KB row-tile load across 4 DMA queues
            eng = eng or nc.gpsimd
            for s4 in range(4):
                eng.dma_start(dst[:, i, 192 * s4:192 * (s4 + 1)],
                              x_d[r0:r0 + 128, 192 * s4:192 * (s4 + 1)])

        # ---- prefetch first group's x ahead of the weight DMAs ----
        x2b_first = x2pool.tile([128, 4, D], F32, tag="x2b", name="x2b_0")
        for i in range(4):
            load_x_fn(x2b_first, i, 128 * i, eng=nc.sync)

        # b2 broadcast to all 128 partitions (once); the per-tile bias add
        # rides a DVE pass in the (ACT-offloaded) LN2 window instead of K=1
        # PE matmuls in the dense MLP2 window.
        b2bc_sb = wpool.tile([128, D], F32)

        def make_b2bc():
            for n0, n1 in ((0, 512), (512, 768)):
                pb = pmlp.tile([128, n1 - n0], F32, tag="pm", name=f"b2bc_{n0}")
                nc.tensor.matmul(pb[:], ones_sb[:, 0:128], b2r_sb[:, n0:n1],
                                 start=True, stop=True)
                nc.vector.tensor_copy(b2bc_sb[:, n0:n1], pb[:])

        # ---- HAM warmup/filler: dependency-free matmuls on the (idle
        # during batch 0) MLP psum pool keep the PE clock at 8/8 through
        # windows where no real PE work is ready ----
        _warm_n = [0]

        def warm(n):
            w = pmlp.tile([128, 128], F32, tag="pm",
                          name=f"warm_{_warm_n[0]}")
            _warm_n[0] += 1
            for _ in range(n):
                nc.tensor.matmul(w[:], ident_sb[:], ident_sb[:],
                                 start=True, stop=True)

        warm(64)

        # ---- weights / constants ----
        wq_sb = wpool.tile([128, NDC, 80], F8)
        wk_sb = wpool.tile([128, NDC, 96], F8)
        wv_sb = wpool.tile([128, NDC, 96], F8)
        wo_sb = wpool.tile([73, D], BF)
        emat_sb = wpool.tile([128, 3, 108], BF)
        cmap_sb = wpool.tile([108, 72], BF)
        w1_sb = wpool.tile([128, NDC, DFF], F8)
        w2_sb = wpool.tile([128, NFC, D], BF)
        b1_sb = wpool.tile([128, NFC], F32)
        b2r_sb = wpool.tile([1, D], BF)
        for sb_t, d_t in ((wq_sb, wq_d), (wk_sb, wk_d), (wv_sb, wv_d),
                          (wo_sb, wo_d), (emat_sb, emat_d), (b1_sb, b1_d),
                          (b2r_sb, b2_d)):
            nc.gpsimd.dma_start(sb_t[:], d_t[:])
        nc.gpsimd.dma_start(cmap_sb[96:108, :], cmap_d[:])
        # big MLP weights: chunked DMAs on the gpsimd-issued queues so the
        # latency-critical small DMAs (x, qk scatter) keep the sync queues
        for c in range(NDC):
            nc.gpsimd.dma_start(w1_sb[:, c, :], w1_d[:, c, :])
        for m in range(NFC):
            nc.gpsimd.dma_start(w2_sb[:, m, :], w2_d[:, m, :])

        def layernorm_group(b, x2b, xT_b, load_x, evict_act,
                            per_tile=False, warm_every=0):
            """LN over group b's 4 token tiles; writes transposed xT_b (fp8).

            The LN affine is folded into the consumer weights host-side, so
            the PSUM->SBUF eviction of each PE transpose is a plain copy
            (paired chunks, on ACT when evict_act to offload the DVE).
            """
            mv = stp.tile([128, 4, 2], F32, tag="mv")
            rstd = stp.tile([128, 4], F32, tag="rstd")
            if evict_act:
                nmr = stp.tile([128, 4], F32, tag="nmr")

            def stats_tile(i):
                if load_x:
                    load_x_fn(x2b, i, (4 * b + i) * 128)
                stats = stp.tile([128, 2, 6], F32, tag="bn")
                for s in range(2):
                    nc.vector.bn_stats(stats[:, s, :],
                                       x2b[:, i, 384 * s:384 * (s + 1)])
                nc.vector.bn_aggr(mv[:, i, :], stats[:])

            def rstd_tile(i):
                nc.scalar.activation(rstd[:, i:i + 1], mv[:, i, 1:2],
                                     AF.Sqrt, bias=eps_sb[:])
                nc.vector.reciprocal(rstd[:, i:i + 1], rstd[:, i:i + 1])
                if evict_act:
                    nc.vector.tensor_tensor(nmr[:, i:i + 1], mv[:, i, 0:1],
                                            rstd[:, i:i + 1],
                                            mybir.AluOpType.mult)
                    nc.vector.tensor_scalar_mul(nmr[:, i:i + 1],
                                                nmr[:, i:i + 1], -1.0)

            def evict_tile(i):
                xn = lnp.tile([128, D], BF, tag="xn")
                if evict_act:
                    nc.scalar.activation(xn[:], x2b[:, i, :], AF.Identity,
                                         bias=nmr[:, i:i + 1],
                                         scale=rstd[:, i:i + 1])
                else:
                    nc.vector.tensor_scalar(
                        out=xn[:], in0=x2b[:, i, :],
                        scalar1=mv[:, i, 0:1], scalar2=rstd[:, i:i + 1],
                        op0=ALU.subtract, op1=ALU.mult)
                for cp in range(NDC // 2):
                    pt = pmlp.tile([128, 2, 128], BF, tag="pm")
                    for half in range(2):
                        c = 2 * cp + half
                        nc.tensor.transpose(pt[:, half, :],
                                            xn[:, 128 * c:128 * (c + 1)],
                                            ident_sb[:])
                    dst = xT_b[:, 2 * cp:2 * cp + 2, 128 * i:128 * (i + 1)]
                    if evict_act:
                        nc.scalar.activation(dst, pt[:], AF.Copy)
                    else:
                        nc.vector.tensor_copy(dst, pt[:])

            if per_tile:
                # fully per-tile: tile i's transposes are gated only by tile
                # i's own stat chain (no in-order-stream wait on tile 3)
                for i in range(4):
                    stats_tile(i)
                    rstd_tile(i)
                    evict_tile(i)
            else:
                for i in range(4):
                    stats_tile(i)
                nc.scalar.activation(rstd[:], mv[:, :, 1], AF.Sqrt,
                                     bias=eps_sb[:])
                nc.vector.reciprocal(rstd[:], rstd[:])
                if evict_act:
                    nc.vector.tensor_tensor(nmr[:], mv[:, :, 0], rstd[:],
                                            mybir.AluOpType.mult)
                    nc.vector.tensor_scalar_mul(nmr[:], nmr[:], -1.0)
                for i in range(4):
                    evict_tile(i)
                if warm_every:
                    # batch-0 startup: fill the PE while the next tile's LN
                    # stat chain (DVE) runs
                    warm(warm_every)

        x2b_t = [None] * BPC
        xnT_t = [None] * BPC

        def ln1(b):
            if b == 0:
                x2b_t[b] = x2b_first
            else:
                x2b_t[b] = x2pool.tile([128, 4, D], F32, tag="x2b",
                                       name=f"x2b_{b}")
            xnT_t[b] = xnt1.tile([128, NDC, 512], F8, tag="xnT1", name=f"xnT_{b}")
            layernorm_group(b, x2b_t[b], xnT_t[b], load_x=(b != 0),
                            evict_act=False, per_tile=(b == 0))

        st = {}  # per-batch attention state

        def qk(b):
            # Packed projection (one [D,72] matmul per tensor), then an
            # SBUF->SBUF DMA scatter into the 32-partition-aligned padded
            # layout the row-tiled score matmuls need.
            xnT = xnT_t[b]
            qT = apool.tile([128, 3, 512], F8, tag="qT", name=f"qT_{b}")
            kT = apool.tile([128, 3, 512], F8, tag="kT", name=f"kT_{b}")
            for w_sb, dst, nm, ew in ((wq_sb, qT, "q", 6),
                                      (wk_sb, kT, "k", 7)):
                nw = 12 * ew
                p = pmlp.tile([nw, 512], F32, tag="pm", name=f"pqk_{b}_{nm}")
                for cc in range(NDC // 2):
                    nc.tensor.matmul(p[:], w_sb[:, 2 * cc:2 * cc + 2, 0:nw],
                                     xnT[:, 2 * cc:2 * cc + 2, :],
                                     start=(cc == 0), stop=(cc == NDC // 2 - 1),
                                     perf_mode=DR)
                pk = spool.tile([nw, 512], F8, tag="qkpk",
                                name=f"qkpk_{b}_{nm}")
                nc.vector.tensor_copy(pk[:], p[:])
                for g in range(3):
                    for j in range(4):
                        h = 4 * g + j
                        nc.sync.dma_start(dst[32 * j:32 * j + ew, g, :],
                                          pk[ew * h:ew * h + ew, :])
                if nm == "q":
                    # ones row per head band (pairs with k's beta-correction
                    # column); engine ops can't address partition 32j+6, DMA can
                    for g in range(3):
                        for j in range(4):
                            nc.sync.dma_start(qT[32 * j + 6:32 * j + 7, g, :],
                                              ones8_sb[:])
            st.setdefault(b, {}).update(qT=qT, kT=kT)
            st[b]["cat"] = pcat_pool.tile([108, 512], F32, tag="cat",
                                          name=f"cat_{b}")
            st[b]["rzb"] = spool.tile([108, 512], BF, tag="rzb",
                                      name=f"rzb_{b}")

        def vmm(b):
            xnT = xnT_t[b]
            vA = apool.tile([128, 4, 84], BF, tag="vA", name=f"vA_{b}")
            for si in range(4):
                p = ppool.tile([128, 84], F32, tag="pb", name=f"pv_{b}_{si}")
                for cc in range(NDC // 2):
                    nc.tensor.matmul(
                        p[:], xnT[:, 2 * cc:2 * cc + 2, si * 128:(si + 1) * 128],
                        wv_sb[:, 2 * cc:2 * cc + 2, 0:84],
                        start=(cc == 0), stop=(cc == NDC // 2 - 1),
                        perf_mode=DR)
                nc.vector.tensor_copy(vA[:, si, :], p[:])
            nc.vector.memset(
                vA[:].rearrange("p s (h e) -> p s h e", e=7)[:, :, :, 6:7], WS)
            st.setdefault(b, {})["vA"] = vA

        def scores_exp(b, g):
            qT, kT = st[b]["qT"], st[b]["kT"]
            expT = epool.tile([128, 4, 1280], BF, tag="expT", name=f"expT_{b}_{g}")
            st[b][f"expT{g}"] = expT
            for si in range(4):
                n = WID[si]
                for j in range(4):
                    pss = ppool.tile([128, 512], F32, tag="pb",
                                     name=f"pss_{b}_{g}_{si}_{j}")
                    nc.tensor.matmul(
                        pss[:, :n],
                        kT[32 * j:32 * j + 7, g, 128 * si:128 * (si + 1)],
                        qT[32 * j:32 * j + 7, g, 128 * si:512],
                        start=True, stop=True,
                        tile_position=(32 * j, 0))
                    o0 = OFFS[si]
                    nc.scalar.activation(expT[:, j, o0:o0 + n],
                                         pss[:, :n], AF.Exp,
                                         scale=1.0 / (WS * WS))
                    nc.gpsimd.tensor_mul(expT[:, j, o0:o0 + 128],
                                         expT[:, j, o0:o0 + 128],
                                         mask_sb[:])
            if tap_exp is not None and b == 0 and g == 0:
                nc.sync.dma_start(tap_exp[:], expT[:])

        def av_perm(b, g):
            # 4 heads col-tiled into the PE array concurrently (32-col bands,
            # each with its own stationary v and moving expT stream); one
            # eviction + one K=128 permutation matmul per group of 4 heads.
            vA, ps_cat = st[b]["vA"], st[b]["cat"]
            expT = st[b][f"expT{g}"]
            po4 = ppool.tile([128, 512], F32, tag="pb", name=f"po4_{b}_{g}")
            for si in range(4):
                for j in range(4):
                    h = 4 * g + j
                    nc.tensor.matmul(
                        po4[32 * j:32 * j + 7, 128 * si:512],
                        vA[:, si, 7 * h:7 * h + 7],
                        expT[:, j, OFFS[si]:OFFS[si] + WID[si]],
                        start=(si == 0), stop=(si == 3),
                        tile_position=(0, 32 * j), skip_group_check=True)
            osb4 = spool.tile([128, 512], BF, tag="osb", name=f"osb_{b}_{g}")
            nc.vector.tensor_copy(osb4[:], po4[:])
            nc.tensor.matmul(ps_cat[:], emat_sb[:, g, :], osb4[:],
                             start=(g == 0), stop=(g == 2),
                             skip_group_check=True)

        def norm_wo(b, prev=None):
            x2b, ps_cat, rzb = x2b_t[b], st[b]["cat"], st[b]["rzb"]
            if tap_cat is not None and b == 0:
                csb = spool.tile([108, 512], F32, tag="csb")
                nc.vector.tensor_copy(csb[:], ps_cat[:])
                nc.sync.dma_start(tap_cat[:], csb[:])
            with nc.allow_low_precision(reason="softmax 1/Z in bf16"):
                nc.vector.reciprocal(rzb[96:108, :], ps_cat[96:108, :])
            if prev is not None:
                mlp2_ti(prev, 0)
                mlp2_ti(prev, 1)
            else:
                warm(40)
            pbc = ppool.tile([72, 512], F32, tag="pb", name=f"pbc_{b}")
            nc.tensor.matmul(pbc[:], cmap_sb[96:108, :], rzb[96:108, :],
                             start=True, stop=True, tile_position=(96, 0))
            bc_sb = spool.tile([72, 512], BF, tag="bc", name=f"bc_{b}")
            nc.vector.tensor_copy(bc_sb[:], pbc[:])
            onT = apool.tile([73, 512], BF, tag="onT", name=f"onT_{b}")
            nc.vector.tensor_mul(onT[0:72, :], ps_cat[0:72, :], bc_sb[:])
            nc.sync.dma_start(onT[72:73, :], ones_sb[:])
            if tap_onT is not None and b == 0:
                nc.sync.dma_start(tap_onT[:], onT[:])
            if prev is not None:
                mlp2_ti(prev, 2)
            for ti in range(4):
                pa = ppool.tile([128, 512], F32, tag="pb", name=f"pwa_{b}_{ti}")
                pb2 = ppool.tile([128, 256], F32, tag="pb", name=f"pwb_{b}_{ti}")
                nc.tensor.matmul(pa[:], onT[:, 128 * ti:128 * (ti + 1)],
                                 wo_sb[:, 0:512], start=True, stop=True)
                nc.tensor.matmul(pb2[:], onT[:, 128 * ti:128 * (ti + 1)],
                                 wo_sb[:, 512:768], start=True, stop=True)
                nc.vector.tensor_add(x2b[:, ti, 0:512], pa[:],
                                     x2b[:, ti, 0:512])
                nc.vector.tensor_add(x2b[:, ti, 512:768], pb2[:],
                                     x2b[:, ti, 512:768])
            if prev is not None:
                mlp2_ti(prev, 3)
            if tap_x2a is not None:
                for i in range(4):
                    r0 = (4 * b + i) * 128
                    nc.sync.dma_start(tap_x2a[r0:r0 + 128, :], x2b[:, i, :])

        def ln2(b):
            xn2T = xnt2.tile([128, NDC, 512], F8, tag="xnT2", name=f"xn2T_{b}")
            st[b]["xn2T"] = xn2T
            layernorm_group(b, x2b_t[b], xn2T, load_x=False,
                            evict_act=True)
            # after LN2 consumed x2b, pre-add b2 so mlp2's eviction is a
            # plain residual add
            for i in range(4):
                nc.vector.tensor_add(x2b_t[b][:, i, :], x2b_t[b][:, i, :],
                                     b2bc_sb[:])

        def mlp1_part(b, mlo, mhi, act=False):
            xn2T = st[b]["xn2T"]
            if "h1T" not in st[b]:
                st[b]["h1T"] = h1pool.tile([128, NFC, 512], BF, tag="h1T",
                                           name=f"h1T_{b}")
            h1T = st[b]["h1T"]
            for m in range(mlo, mhi):
                p = pmlp.tile([128, 512], F32, tag="pm", name=f"pm1_{b}_{m}")
                for cc in range(NDC // 2):
                    nc.tensor.matmul(
                        p[:], w1_sb[:, 2 * cc:2 * cc + 2, 128 * m:128 * (m + 1)],
                        xn2T[:, 2 * cc:2 * cc + 2, :],
                        start=(cc == 0), stop=(cc == NDC // 2 - 1),
                        perf_mode=DR)
                if act:
                    nc.scalar.activation(h1T[:, m, :], p[:], AF.Relu,
                                         bias=b1_sb[:, m:m + 1])
                else:
                    # relu on DVE: ACT is saturated by exp in this window
                    nc.vector.tensor_scalar(
                        out=h1T[:, m, :], in0=p[:],
                        scalar1=b1_sb[:, m:m + 1], scalar2=0.0,
                        op0=ALU.add, op1=ALU.max)

        def mlp2_ti(b, ti):
            x2b, h1T = x2b_t[b], st[b]["h1T"]
            if True:
                tix = 4 * b + ti
                pa = pmlp.tile([128, 512], F32, tag="pm", name=f"p2a_{b}_{ti}")
                pb2 = pmlp.tile([128, 256], F32, tag="pm", name=f"p2b_{b}_{ti}")
                for m in range(NFC):
                    nc.tensor.matmul(pa[:],
                                     h1T[:, m, 128 * ti:128 * (ti + 1)],
                                     w2_sb[:, m, 0:512],
                                     start=(m == 0), stop=(m == NFC - 1))
                    nc.tensor.matmul(pb2[:],
                                     h1T[:, m, 128 * ti:128 * (ti + 1)],
                                     w2_sb[:, m, 512:768],
                                     start=(m == 0), stop=(m == NFC - 1))
                nc.vector.tensor_add(x2b[:, ti, 0:512], pa[:],
                                     x2b[:, ti, 0:512])
                nc.vector.tensor_add(x2b[:, ti, 512:768], pb2[:],
                                     x2b[:, ti, 512:768])
                for s4 in range(4):
                    nc.gpsimd.dma_start(
                        out_d[128 * tix:128 * (tix + 1),
                              192 * s4:192 * (s4 + 1)],
                        x2b[:, ti, 192 * s4:192 * (s4 + 1)])

        # ---- software-pipelined schedule: MLP(b-1) and the next batch's
        # LN1/QKV interleaved into attention(b) so the PE in-order stream
        # always has independent matmul work during exp/DVE waits ----
        ln1(0)
        qk(0)
        for b in range(BPC):
            if b > 1:
                vmm(b)
            elif b == 0:
                make_b2bc()
            if tap_xnT is not None and b == 0:
                nc.sync.dma_start(tap_xnT[:], xnT_t[0][:])
            if b + 1 < BPC:
                ln1(b + 1)
            for g in range(3):
                scores_exp(b, g)
                if b >= 1:
                    mlp1_part(b - 1, 8 * g, 8 * (g + 1))
                else:
                    warm(64)
                    if g == 0:
                        vmm(0)
                    elif g == 1:
                        vmm(1)
                if g == 2 and b + 1 < BPC:
                    # next batch's projections: the scatter DMAs drain during
                    # the norm/ln2/mlp2 window instead of stalling scores
                    qk(b + 1)
                av_perm(b, g)
            norm_wo(b, prev=(b - 1 if b >= 1 else None))
            ln2(b)
        mlp1_part(BPC - 1, 0, NFC, act=True)
        for ti in range(4):
            mlp2_ti(BPC - 1, ti)

    nc.compile()
    return nc, t


def prepare_inputs(inputs):
    """Host-side: cast/pad/reshape weights into the kernel's layouts.

    fp8(e4m3) weights are pre-scaled by WS=64 to clear the subnormal range
    (sigma≈0.02 raw); the scales cancel exactly on-device: q·k through the
    exp scale=1/WS^2, v through the WS-valued aug column and 1/Z, W1
    through W2/WS (h1 is stored as WS·relu-units).
    """
    f = lambda k: np.asarray(inputs[k], np.float32)
    Wq, Wk, Wv, Wo = f("Wq"), f("Wk"), f("Wv"), f("Wo")
    g1v, be1v = f("g1"), f("beta1")
    g2v, be2v = f("g2"), f("beta2")
    cast = lambda a: np.ascontiguousarray(a.astype(npBF))
    cast8 = lambda a: np.ascontiguousarray(
        np.clip(a, -240.0, 240.0).astype(npF8))

    # LN affines folded into the consuming weights (z = normalized tokens):
    #   xn@W = z@(diag(g1)W) + beta1@W.  The q-side bias only shifts each
    #   softmax row by a constant (exp(c) cancels through o/Z); the k-side
    #   bias term bq.k~[s] rides a 7th k column (paired with a q ones row);
    #   the v bias passes through softmax intact -> folded into Wo's bias.
    Wq_eff = Wq * g1v[None, :, None]
    Wk_eff = Wk * g1v[None, :, None]
    Wv_eff = Wv * g1v[None, :, None]

    def qk_pack(W, scale):
        # [H, D, E] -> [D, H*E] packed columns, tiled to [128, NDC, 80]
        # (padded to 80 so the fp8 DoubleRow pair-stride is 16B-aligned)
        Wp = np.zeros((D, 80), np.float32)
        Wp[:, 0:H * E] = np.transpose(W, (1, 0, 2)).reshape(D, H * E) * scale
        return cast8(Wp.reshape(NDC, 128, 80).transpose(1, 0, 2))

    wq = qk_pack(Wq_eff, WS * E ** -0.5)
    Wk_aug = np.zeros((D, 96), np.float32)
    for h in range(H):
        bq_h = be1v @ Wq[h]
        Wk_aug[:, 7 * h:7 * h + 6] = Wk_eff[h] * WS
        Wk_aug[:, 7 * h + 6] = (Wk_eff[h] @ bq_h) * (WS * WS * E ** -0.5)
    wk = cast8(Wk_aug.reshape(NDC, 128, 96).transpose(1, 0, 2))
    Wv_aug = np.zeros((D, 96), np.float32)
    for h in range(H):
        Wv_aug[:, 7 * h:7 * h + 6] = Wv_eff[h] * WS
    wv = cast8(Wv_aug.reshape(NDC, 128, 96).transpose(1, 0, 2))
    wo = np.zeros((73, D), np.float32)
    wo[0:72] = Wo
    bv_cat = np.concatenate([be1v @ Wv[h] for h in range(H)])
    wo[72] = f("bo") + bv_cat @ Wo
    # emat: [128, 3, 108] 0/1 permutation; row 32j+e of group g routes
    # head (4g+j)'s o row e -> packed row 6h+e, and row 32j+6 routes the
    # softmax denominator Z -> row 96+h.
    emat = np.zeros((128, 3, 108), np.float32)
    cmap = np.zeros((H, 72), np.float32)
    for h in range(H):
        g, j = divmod(h, 4)
        for e in range(6):
            emat[32 * j + e, g, 6 * h + e] = 1.0
            cmap[h, 6 * h + e] = 1.0
        emat[32 * j + 6, g, 96 + h] = 1.0
    W1_eff = f("W1") * g2v[:, None]
    b1_eff = be2v @ f("W1") + f("b1")
    w1 = cast8((W1_eff * WS).reshape(NDC, 128, DFF).transpose(1, 0, 2))
    w2 = cast((f("W2") / WS).reshape(NFC, 128, D).transpose(1, 0, 2))
    b1 = np.ascontiguousarray((b1_eff * WS).reshape(NFC, 128).T)
    shared = dict(wq=wq, wk=wk, wv=wv, wo=cast(wo), emat=cast(emat),
                  cmap=cast(cmap), w1=w1, w2=w2, b1=b1,
                  b2r=cast(f("b2").reshape(1, D)))
    x = f("x")
    in_maps = []
    for c in range(NCORES):
        m = dict(shared)
        m["x"] = np.ascontiguousarray(
            x[c * BPC:(c + 1) * BPC].reshape(TT, D))
        in_maps.append(m)
    return in_maps


def kernel(**inputs):
    from concourse.bass_utils import run_bass_kernel_spmd
    key = "prog"
    if key not in _PROG_CACHE:
        _PROG_CACHE[key] = build_program()
    nc, _ = _PROG_CACHE[key]
    in_maps = prepare_inputs(inputs)
    trace = bool(int(os.environ.get("KERNEL_TRACE", "0")))
    res = run_bass_kernel_spmd(nc, in_maps, list(range(NCORES)), trace=trace)
    if trace and res.exec_time_ns is not None:
        print(f"HW exec time: {res.exec_time_ns} ns")
        _PROG_CACHE["last_exec_ns"] = res.exec_time_ns
        _PROG_CACHE["last_results"] = res
    out = np.empty((B, T, D), np.float32)
    for c in range(NCORES):
        out[c * BPC:(c + 1) * BPC] = res.results[c]["out"].reshape(BPC, T, D)
    return out

